# revision 11
# baseline (speedup 1.0000x reference)
"""Trainium2 Bass kernel v6 for nn_Attention_30666066493686.

Region-attention over N=36 regions:
  hidden = tanh(region @ Wr + frame @ Wf + b_att)          [T,N,B,A]
  att    = hidden . W_full  (+ b_full, dropped: softmax-shift invariant)
  alpha  = softmax_n(where(mask, -1e9, att))
  out    = sum_n alpha * region                            [T,B,D]

Sharding: data-parallel over T across 8 NeuronCores (4 timesteps each);
params replicated; no collectives.

v6 = v5 + host-side mask compaction. Masked rows have alpha == 0
exactly, and the mask is a host-visible input, so the host drops masked
(n, b) rows before upload: both region streams shrink by the mask
density (~50%), which is the whole DMA bottleneck. The regular
(n, b) <-> (partition, chunk) mapping is replaced by host-built 0/1
indicator matrices:
  - indB [p, b] per chunk: one-hot of b(row); drives the softmax
    denominator fold (PE matmul) and the diag-expanded phase-2 weights
    (DVE scale), zero on padding rows.
  - ind2 [b, col] per timestep (= indB transposed): routes fproj[b, :]
    into hidden^T columns via one accumulating PE matmul (and adds 0 on
    padding columns).
Phase 1 runs on fp8e4m3 region/Wr with DoubleRow (2 k-chunks per
matmul); phase 2 and everything alpha-facing stays bf16. Softmax skips
max-subtraction (|att| is small); 1/S folds into the output scale.

Queue discipline: region loads + out stores ride the SP HWDGE FIFO in
need-order (rT0..rT3 then rN0..rN3, outs deferred last); constants ride
the ACT FIFO so nothing head-of-line-blocks tanh or the loads.
"""

import ml_dtypes
import numpy as np

T, N, B, D, A = 32, 36, 64, 512, 128
N_CORES = 8
T_LOC = T // N_CORES           # 4
ROWS = N * B                   # 2304
NJ = D // 128                  # 4

# const blob column layout (bf16): Wf | wfull | frameT | b_att
CB_WF = 0
CB_WFULL = 512
CB_FRAMET = 513
CB_BATT = 1537
CB_W = 1538

_NC_CACHE = {}


def _groups(rc):
    out = []
    c0 = 0
    while c0 < rc:
        out.append((c0, min(512, rc - c0)))
        c0 += 512
    return out


def _build_nc(iters=1, nchc=10, unroll=1):
    import concourse.bacc as bacc
    from concourse import mybir
    from concourse.tile import TileContext

    f32 = mybir.dt.float32
    bf16 = mybir.dt.bfloat16
    fp8 = mybir.dt.float8e4
    AF = mybir.ActivationFunctionType
    rc = nchc * 128

    nc = bacc.Bacc(
        "TRN2", target_bir_lowering=False, debug=False, num_devices=N_CORES
    )
    regionTc = nc.dram_tensor("regionTc", [T_LOC, 128, NJ * rc], fp8, kind="ExternalInput")
    regionNc = nc.dram_tensor("regionNc", [T_LOC, 128, nchc * 512], bf16, kind="ExternalInput")
    cb16 = nc.dram_tensor("cb16", [128, CB_W], bf16, kind="ExternalInput")
    watt8 = nc.dram_tensor("watt8", [128, 512], fp8, kind="ExternalInput")
    indB = nc.dram_tensor("indB", [128, T_LOC * nchc * 64], fp8, kind="ExternalInput")
    ind2 = nc.dram_tensor("ind2", [64, T_LOC * rc], fp8, kind="ExternalInput")
    out = nc.dram_tensor("out", [T_LOC, B, D], f32, kind="ExternalOutput")

    with TileContext(nc) as tc:
        with (
            tc.tile_pool(name="consts", bufs=1) as consts,
            tc.tile_pool(name="rtp", bufs=4) as rtp,
            tc.tile_pool(name="rnp", bufs=4) as rnp,
            tc.tile_pool(name="thp", bufs=2) as thp,
            tc.tile_pool(name="smallp", bufs=4) as smallp,
            tc.tile_pool(name="diagp", bufs=3) as diagp,
            tc.tile_pool(name="outp", bufs=4) as outp,
            tc.tile_pool(name="phh", bufs=4, space="PSUM") as phh,
            tc.tile_pool(name="psmall", bufs=2, space="PSUM") as psmall,
            tc.tile_pool(name="po", bufs=2, space="PSUM") as po,
        ):
            # ---- constants: packed DMAs on the ACT queue ----
            cb = consts.tile([128, CB_W], bf16)
            nc.scalar.dma_start(out=cb, in_=cb16.ap())
            w8 = consts.tile([128, 2, 2, 128], fp8)
            nc.scalar.dma_start(
                out=w8, in_=watt8.ap().rearrange("p (kp kk a) -> p kp kk a", kp=2, kk=2)
            )
            indB_sb = consts.tile([128, T_LOC, nchc, 64], fp8)
            nc.scalar.dma_start(
                out=indB_sb,
                in_=indB.ap().rearrange("p (t c b) -> p t c b", t=T_LOC, c=nchc),
            )
            ind2_sb = consts.tile([64, T_LOC, rc], fp8)
            nc.scalar.dma_start(
                out=ind2_sb, in_=ind2.ap().rearrange("p (t r) -> p t r", t=T_LOC)
            )

            def wf_sb(j):  # [128, 128] chunk j of Wf (frame half of W_att)
                return cb[:, CB_WF + j * 128 : CB_WF + (j + 1) * 128]

            wfull_sb = cb[:, CB_WFULL : CB_WFULL + 1]

            def frameT_sb(j):
                return cb[:, CB_FRAMET + j * 256 : CB_FRAMET + (j + 1) * 256]

            batt_col = cb[:, CB_BATT : CB_BATT + 1]

            # ---- preamble: fproj[b, A] per t = (frame @ Wf)[t] ----
            fproj_sb = consts.tile([64, T_LOC, 128], bf16)
            for t in range(T_LOC):
                pf = psmall.tile([64, 128], f32, tag="s", name=f"pf{t}")
                for j in range(NJ):
                    nc.tensor.matmul(
                        pf,
                        lhsT=frameT_sb(j)[:, t * 64 : (t + 1) * 64],
                        rhs=wf_sb(j),
                        start=(j == 0),
                        stop=(j == NJ - 1),
                    )
                nc.scalar.copy(out=fproj_sb[:, t, :], in_=pf)

            # ---- per-timestep body ----
            def body(_iv=None, copies=1):
                per = [({}, {}, {}, []) for _ in range(copies)]

                def load_rT(st, t, split):
                    rTs, rNs, states, osbs = st
                    rT = rtp.tile([128, NJ, rc], fp8, tag="rT", name=f"rT{t}")
                    rTd = regionTc.ap()[t].rearrange("p (j r) -> p j r", j=NJ)
                    half = (rc // 2 + 63) // 64 * 64
                    pieces = ((0, half), (half, rc)) if split else ((0, rc),)
                    for lo, hi in pieces:
                        nc.sync.dma_start(out=rT[:, :, lo:hi], in_=rTd[:, :, lo:hi])
                    rTs[t] = rT

                def load_rN(st, t, split_all, split_tail):
                    rTs, rNs, states, osbs = st
                    rN = rnp.tile([128, nchc, 512], bf16, tag="rN", name=f"rN{t}")
                    rNd = regionNc.ap()[t].rearrange("p (c d) -> p c d", c=nchc)
                    third = max(1, nchc // 3)
                    if split_all:
                        cuts = [0, third, 2 * third, nchc]
                    else:
                        cuts = [0, nchc]
                    if split_tail and t == T_LOC - 1 and nchc > 2:
                        cuts = [0, third, 2 * third, nchc - 1, nchc]
                    for lo, hi in zip(cuts[:-1], cuts[1:]):
                        if lo < hi:
                            nc.sync.dma_start(out=rN[:, lo:hi, :], in_=rNd[:, lo:hi, :])
                    rNs[t] = rN

                def p1(st, t):
                    rTs, rNs, states, osbs = st
                    # phase 1: hidden^T[A, rows] = Wr^T @ region^T (fp8
                    # DoubleRow) + fproj routed via ind2; tanh adds b_att
                    rT = rTs[t]
                    th = thp.tile([128, rc], bf16, tag="th", name=f"th{t}")
                    for g, (c0, cw) in enumerate(_groups(rc)):
                        ph = phh.tile([128, 512], f32, tag="phh", name=f"ph{t}_{g}")
                        for jp in range(2):
                            nc.tensor.matmul(
                                ph[:, :cw],
                                lhsT=w8[:, jp],
                                rhs=rT[:, 2 * jp : 2 * jp + 2, c0 : c0 + cw],
                                start=(jp == 0),
                                stop=False,
                                perf_mode=mybir.MatmulPerfMode.DoubleRow,
                            )
                        nc.tensor.matmul(
                            ph[:, :cw],
                            lhsT=fproj_sb[:, t, :],
                            rhs=ind2_sb[:, t, c0 : c0 + cw],
                            start=False,
                            stop=True,
                        )
                        nc.scalar.activation(
                            out=th[:, c0 : c0 + cw],
                            in_=ph[:, :cw],
                            func=AF.Tanh,
                            bias=batt_col,
                        )
                    # att columns + masked-softmax denominator
                    patt = psmall.tile([128, nchc], f32, tag="s", name=f"pa{t}")
                    for c in range(nchc):
                        nc.tensor.matmul(
                            patt[:, c : c + 1],
                            lhsT=th[:, c * 128 : (c + 1) * 128],
                            rhs=wfull_sb,
                            start=True,
                            stop=True,
                        )
                    expf = smallp.tile([128, nchc], f32, tag="expf", name=f"ef{t}")
                    nc.scalar.activation(out=expf, in_=patt, func=AF.Exp)
                    expb = smallp.tile([128, nchc], bf16, tag="expb", name=f"ex{t}")
                    nc.vector.tensor_copy(out=expb, in_=expf)
                    psS = psmall.tile([64, 1], f32, tag="s", name=f"ps{t}")
                    for c in range(nchc):
                        nc.tensor.matmul(
                            psS,
                            lhsT=indB_sb[:, t, c, :],
                            rhs=expb[:, c : c + 1],
                            start=(c == 0),
                            stop=(c == nchc - 1),
                        )
                    rs = smallp.tile([64, 1], f32, tag="rs", name=f"rs{t}")
                    nc.vector.reciprocal(out=rs, in_=psS)
                    states[t] = (expf, rs)

                def p2(st, t):
                    rTs, rNs, states, osbs = st
                    # phase 2: out[b, :] = (sum_c diag-expand(e)_c^T @ rN_c)/S
                    expf, rs = states[t]
                    rN = rNs[t]
                    po_t = po.tile([64, 512], f32, tag="po", name=f"po{t}")
                    for c in range(nchc):
                        dg = diagp.tile([128, 64], bf16, tag="dg", name=f"dg{t}_{c}")
                        nc.vector.tensor_scalar_mul(
                            out=dg, in0=indB_sb[:, t, c, :], scalar1=expf[:, c : c + 1]
                        )
                        nc.tensor.matmul(
                            po_t,
                            lhsT=dg,
                            rhs=rN[:, c, :],
                            start=(c == 0),
                            stop=(c == nchc - 1),
                        )
                    osb = outp.tile([64, 512], f32, tag="osb", name=f"ob{t}")
                    nc.scalar.activation(out=osb, in_=po_t, func=AF.Copy, scale=rs)
                    osbs.append((t, osb))

                for u, st in enumerate(per):
                    for t in range(T_LOC):
                        load_rT(st, t, split=True)
                    for t in range(T_LOC):
                        load_rN(st, t, split_all=True, split_tail=(u == copies - 1))
                for st in per:
                    for t in range(T_LOC):
                        p1(st, t)
                    for t in range(T_LOC):
                        p2(st, t)
                # out-stores issued after every region-load issue so the SP
                # FIFO never head-of-line-blocks a later load
                for st in per:
                    for t, osb in st[3]:
                        nc.sync.dma_start(out=out.ap()[t], in_=osb)

            if iters == 1:
                body()
            else:
                assert iters % unroll == 0, (iters, unroll)
                with tc.For_i(
                    0, iters // unroll, 1, hint_engines=(mybir.EngineType.PE,)
                ) as iv:
                    body(iv, copies=unroll)

    nc.compile()
    return nc


def _get_nc(iters=1, nchc=10, unroll=None):
    if unroll is None:
        unroll = 4 if iters > 1 and iters % 4 == 0 else 1
    key = (iters, nchc, unroll)
    if key not in _NC_CACHE:
        _NC_CACHE[key] = _build_nc(iters, nchc, unroll)
    return _NC_CACHE[key]


def _nchc_for(mask):
    keep = ~np.asarray(mask, bool).reshape(T, ROWS)
    counts = keep.sum(axis=1)
    return max(1, int(-(-int(counts.max()) // 128)))


def _make_in_maps(region_feat, frame_feat, mask, W_att, b_att, W_full, nchc=None):
    bf16 = ml_dtypes.bfloat16
    fp8 = ml_dtypes.float8_e4m3
    mask = np.asarray(mask, bool)
    if nchc is None:
        nchc = _nchc_for(mask)
    rc = nchc * 128

    region_f = np.asarray(region_feat, np.float32)        # [T, N, B, D]
    frame_b = np.asarray(frame_feat).astype(bf16)         # [T, B, D]

    w8 = np.ascontiguousarray(
        np.asarray(W_att)[:512]
        .astype(fp8)
        .reshape(4, 128, 128)
        .transpose(1, 0, 2)
        .reshape(128, 512)
    )

    in_maps = []
    for cidx in range(N_CORES):
        sl = slice(cidx * T_LOC, (cidx + 1) * T_LOC)
        regc = np.zeros((T_LOC, rc, D), np.float32)
        ohe = np.zeros((T_LOC, rc, 64), np.float32)
        for tt in range(T_LOC):
            kept = np.flatnonzero(~mask[sl][tt].reshape(ROWS))
            nk = len(kept)
            regc[tt, :nk] = region_f[sl][tt].reshape(ROWS, D)[kept]
            ohe[tt, np.arange(nk), kept % 64] = 1.0
        regT = np.ascontiguousarray(
            regc.astype(fp8)
            .reshape(T_LOC, rc, NJ, 128)
            .transpose(0, 3, 2, 1)
            .reshape(T_LOC, 128, NJ * rc)
        )
        regN = np.ascontiguousarray(
            regc.astype(bf16)
            .reshape(T_LOC, nchc, 128, 512)
            .transpose(0, 2, 1, 3)
            .reshape(T_LOC, 128, nchc * 512)
        )
        iB = np.ascontiguousarray(
            ohe.reshape(T_LOC, nchc, 128, 64)
            .transpose(2, 0, 1, 3)
            .reshape(128, T_LOC * nchc * 64)
        ).astype(fp8)
        i2 = np.ascontiguousarray(
            ohe.transpose(2, 0, 1).reshape(64, T_LOC * rc)
        ).astype(fp8)
        frm = frame_b[sl].reshape(T_LOC * B, NJ, 128)     # [tb, j, dd]
        frmT = frm.transpose(2, 1, 0).reshape(128, NJ * 256)
        cb = np.zeros((128, CB_W), bf16)
        cb[:, CB_WF : CB_WF + 512] = (
            np.asarray(W_att)[512:].reshape(4, 128, 128).transpose(1, 0, 2).reshape(128, 512)
        ).astype(bf16)
        cb[:, CB_WFULL] = np.asarray(W_full).astype(bf16)
        cb[:, CB_FRAMET : CB_FRAMET + 1024] = frmT
        cb[:, CB_BATT] = np.asarray(b_att).astype(bf16)
        in_maps.append(
            {
                "regionTc": regT,
                "regionNc": regN,
                "cb16": np.ascontiguousarray(cb),
                "watt8": w8,
                "indB": iB,
                "ind2": i2,
            }
        )
    return in_maps


def kernel(region_feat, frame_feat, mask, W_att, b_att, W_full, b_full=None):
    """Full-input entry point. b_full is accepted but unused: softmax is
    invariant to a constant shift of the logits."""
    from concourse.bass_utils import run_bass_kernel_spmd

    nchc = _nchc_for(mask)
    nc = _get_nc(1, nchc)
    in_maps = _make_in_maps(
        region_feat, frame_feat, mask, W_att, b_att, W_full, nchc=nchc
    )
    res = run_bass_kernel_spmd(nc, in_maps, core_ids=list(range(N_CORES)))
    return np.concatenate(
        [res.results[c]["out"] for c in range(N_CORES)], axis=0
    ).astype(np.float32)
